# revision 49
# baseline (speedup 1.0000x reference)
"""Trainium2 Bass kernel for nn_GaussianMixture (mixture-of-5-Gaussians sampler).

Strategy: data-parallel over the row dim N=16384 across 8 NeuronCores
(2048 rows/core), MLP weights replicated. Single pass over all 2048 rows
per core.

The probs MLP needs near-fp32 logits (the Gumbel argmax flips components
for logit errors ~1e-4, and each flip costs O(1) error on that row), so
its two hidden layers run as 3-matmul bf16 hi/lo splits (Whi*xhi +
Whi*xlo + Wlo*xhi, ~17 effective mantissa bits; measured logit absmax
err 3.7e-5 vs a 1.24e-4 min top-2 score margin on this dataset). c and
the probs weights arrive from the host pre-transposed and pre-split into
bf16 hi + bf16 residual parts; the hidden activation h0 is split
on-device with a bf16 round + subtract.

The expert MLPs run plain bf16 (measured ~216 ns per 512-row matmul vs
233 ns for f32r on this silicon; output error stays ~3e-3 << the 2e-2
budget, and expert precision cannot flip the component choice). The
experts reuse the bf16 hi part of c as their input.

All weights arrive host-pre-arranged into partition-contiguous
[128, ...] layout so every weight DMA is a trivial descriptor — complex
rearrange patterns cost the SP engine ~10us of serial descriptor
generation per expert, which showed up as PE stalls. Expert 0's full
weight set is prefetched during probs.

Activations stay feature-major ([h_features, n_rows]) through the hidden
layers so no transposes are needed; the final expert layer uses the
feature-major hidden state as lhsT to produce row-major output directly,
so noise / sampling / weighted-combine all run row-major with
per-partition scalar weights. Each row-tile's output is DMA'd out as
soon as the last expert's contribution lands.
"""
import sys

sys.path.insert(0, "/opt/trn_rl_repo")

from contextlib import ExitStack

import numpy as np

import concourse.bass as bass
import concourse.tile as tile
from concourse import bacc, mybir
from concourse.bass_utils import run_bass_kernel_spmd
from concourse.masks import make_identity

F32 = mybir.dt.float32
F32R = mybir.dt.float32r
BF16 = mybir.dt.bfloat16
AF = mybir.ActivationFunctionType
ALU = mybir.AluOpType
AX = mybir.AxisListType

N_CORES = 8
N, CDIM, FDIM, HDIM, K = 16384, 512, 512, 1024, 5
F2 = 2 * FDIM
WEIGHT = 5.0
EPS = 1e-20

CT = CDIM // 128  # 4 c-feature tiles
HT = HDIM // 128  # 8 h-feature tiles


def build_program(nl: int):
    """Build the per-core program for nl rows (nl=2048 for the real run)."""
    assert nl % 512 == 0
    nb = 512              # n-block (matmul moving size)
    nbc = nl // nb        # n-blocks
    ntl = nb // 128       # row-tiles per n-block
    rt = nl // 128        # row-tiles total

    nc = bacc.Bacc("TRN2", target_bir_lowering=False, debug=False)

    # All weight/bias tensors are host-pre-arranged to [128, ...]
    # partition-contiguous layout (see make_in_maps).
    chb_d = nc.dram_tensor("chb", [128, CT, nl], BF16, kind="ExternalInput").ap()
    crb_d = nc.dram_tensor("crb", [128, CT, nl], BF16, kind="ExternalInput").ap()
    noise_d = nc.dram_tensor("noise", [K, nl, FDIM], F32, kind="ExternalInput").ap()
    gu_d = nc.dram_tensor("gumbel_u", [nl, K], F32, kind="ExternalInput").ap()
    pw0h_d = nc.dram_tensor("pw0h", [128, CT, HDIM], BF16, kind="ExternalInput").ap()
    pw0l_d = nc.dram_tensor("pw0l", [128, CT, HDIM], BF16, kind="ExternalInput").ap()
    pw1h_d = nc.dram_tensor("pw1h", [128, HT, HDIM], BF16, kind="ExternalInput").ap()
    pw1l_d = nc.dram_tensor("pw1l", [128, HT, HDIM], BF16, kind="ExternalInput").ap()
    pw2_d = nc.dram_tensor("pw2", [128, HT, K], F32, kind="ExternalInput").ap()
    pbb_d = nc.dram_tensor("pbb", [128, 2, HT], F32, kind="ExternalInput").ap()
    pb2_d = nc.dram_tensor("pb2", [K], F32, kind="ExternalInput").ap()
    gw0_d = nc.dram_tensor("gw0", [K, 128, CT, HDIM], BF16, kind="ExternalInput").ap()
    gw1_d = nc.dram_tensor("gw1", [K, 128, HT, HDIM], BF16, kind="ExternalInput").ap()
    gw2_d = nc.dram_tensor("gw2", [K, 128, HT, F2], BF16, kind="ExternalInput").ap()
    gbb_d = nc.dram_tensor("gbb", [K, 128, 2, HT], F32, kind="ExternalInput").ap()
    gb2_d = nc.dram_tensor("gb2", [K, F2], F32, kind="ExternalInput").ap()
    out_d = nc.dram_tensor("out", [nl, FDIM], F32, kind="ExternalOutput").ap()

    with tile.TileContext(nc) as tc:
        with ExitStack() as gctx:
            const = gctx.enter_context(tc.tile_pool(name="const", bufs=1))
            ps_mm = gctx.enter_context(
                tc.tile_pool(name="ps_mm", bufs=4, space="PSUM")
            )
            ps_l3 = gctx.enter_context(
                tc.tile_pool(name="ps_l3", bufs=2, space="PSUM")
            )
            sb = gctx.enter_context(tc.tile_pool(name="sb", bufs=1))
            pre = gctx.enter_context(tc.tile_pool(name="pre", bufs=1))

            # packed const tile: identity | pb2 broadcast | eps
            constt = const.tile([128, 134], F32, tag="constt")
            ident = constt[:, 0:128]
            pb2_b = constt[:, 128:133]
            eps_b = constt[:, 133:134]
            make_identity(nc, ident)
            nc.gpsimd.dma_start(out=pb2_b, in_=pb2_d.partition_broadcast(128))
            nc.vector.memset(eps_b, EPS)

            # bf16 hi part of c.T: probs l0 input AND the experts' c input.
            # First block arrives as per-ci slices so the very first matmul
            # group can start as soon as ~0.4 MB has landed.
            chb = sb.tile([128, CT, nl], BF16, tag="chb")

            # smalls: logits | gu | lg1 | sc | wgt
            smalls = sb.tile([128, 5, rt, K], F32, tag="smalls")
            logits = smalls[:, 0]
            gu = smalls[:, 1]
            lg1 = smalls[:, 2]
            sc = smalls[:, 3]
            wgt = smalls[:, 4]

            # ---- probs MLP (bf16 hi/lo split, ~fp32-exact logits) ----
            with ExitStack() as pctx:
                pw = pctx.enter_context(tc.tile_pool(name="pw", bufs=1))
                act = pctx.enter_context(tc.tile_pool(name="pact", bufs=1))
                ptmp = pctx.enter_context(tc.tile_pool(name="ptmp", bufs=1))

                # DMA issue order tracks first use: block-0 inputs + l0
                # weights (per-ci so the first group streams), l1 weights,
                # the rest of c, then the expert-0 prefetch.
                crb = pw.tile([128, CT, nl], BF16, tag="crb")
                pw0h_s = pw.tile([128, CT, HDIM], BF16, tag="w0h")
                pw0l_s = pw.tile([128, CT, HDIM], BF16, tag="w0l")
                for ci in range(CT):
                    nc.sync.dma_start(
                        out=chb[:, ci, 0:nb], in_=chb_d[:, ci, 0:nb]
                    )
                    nc.sync.dma_start(
                        out=pw0h_s[:, ci, :], in_=pw0h_d[:, ci, :]
                    )
                    nc.sync.dma_start(
                        out=crb[:, ci, 0:nb], in_=crb_d[:, ci, 0:nb]
                    )
                    nc.sync.dma_start(
                        out=pw0l_s[:, ci, :], in_=pw0l_d[:, ci, :]
                    )
                pw1h_s = pw.tile([128, HT, HDIM], BF16, tag="w1h")
                nc.sync.dma_start(out=pw1h_s, in_=pw1h_d)
                pw1l_s = pw.tile([128, HT, HDIM], BF16, tag="w1l")
                nc.sync.dma_start(out=pw1l_s, in_=pw1l_d)
                for b in range(1, nbc):
                    cs = slice(b * nb, (b + 1) * nb)
                    nc.sync.dma_start(out=chb[:, :, cs], in_=chb_d[:, :, cs])
                    nc.sync.dma_start(out=crb[:, :, cs], in_=crb_d[:, :, cs])
                pw2_s = pw.tile([128, HT, K], F32, tag="w2")
                nc.gpsimd.dma_start(out=pw2_s, in_=pw2_d)
                pbb = pw.tile([128, 2, HT], F32, tag="pbb")
                nc.gpsimd.dma_start(out=pbb, in_=pbb_d)
                # expert 0's full weight set: descriptors + transfer hidden
                # under probs compute
                gw0_f = pre.tile([128, CT, HDIM], BF16, tag="gw0f")
                nc.sync.dma_start(out=gw0_f, in_=gw0_d[0])
                gw1_f = pre.tile([128, HT, HDIM], BF16, tag="gw1f")
                nc.sync.dma_start(out=gw1_f, in_=gw1_d[0])
                gw2_f = pre.tile([128, HT, F2], BF16, tag="gw2f")
                nc.sync.dma_start(out=gw2_f, in_=gw2_d[0])
                gbb_f = pre.tile([128, 2, HT], F32, tag="gbbf")
                nc.sync.dma_start(out=gbb_f, in_=gbb_d[0])
                bb_f = pre.tile([128, FDIM], F32, tag="bbf")
                nc.sync.dma_start(
                    out=bb_f, in_=gb2_d[0, 0:FDIM].partition_broadcast(128)
                )

                for b in range(nbc):
                    cs = slice(b * nb, (b + 1) * nb)
                    h0h = act.tile([128, HT, nb], BF16, tag="h0h")
                    h0l = act.tile([128, HT, nb], BF16, tag="h0l")
                    for ht in range(HT):
                        hs = slice(ht * 128, (ht + 1) * 128)
                        ps = ps_mm.tile([128, nb], F32, tag="mm")
                        for ci in range(CT):
                            nc.tensor.matmul(
                                ps[:], pw0h_s[:, ci, hs], chb[:, ci, cs],
                                start=(ci == 0), stop=False,
                            )
                            nc.tensor.matmul(
                                ps[:], pw0h_s[:, ci, hs], crb[:, ci, cs],
                                start=False, stop=False,
                            )
                            nc.tensor.matmul(
                                ps[:], pw0l_s[:, ci, hs], chb[:, ci, cs],
                                start=False, stop=(ci == CT - 1),
                            )
                        # relu in fp32, then split: hi = bf16(h0), lo = bf16(h0 - hi)
                        h0t = ptmp.tile([128, nb], F32, tag="h0t", bufs=2)
                        nc.scalar.activation(
                            h0t[:], ps[:], AF.Relu, bias=pbb[:, 0, ht : ht + 1]
                        )
                        nc.vector.tensor_copy(h0h[:, ht, :], h0t[:])
                        nc.gpsimd.tensor_sub(h0l[:, ht, :], h0t[:], h0h[:, ht, :])
                    h1 = act.tile([128, HT, nb], F32, tag="h1")
                    for h2 in range(HT):
                        hs = slice(h2 * 128, (h2 + 1) * 128)
                        ps = ps_mm.tile([128, nb], F32, tag="mm")
                        for h1t in range(HT):
                            nc.tensor.matmul(
                                ps[:], pw1h_s[:, h1t, hs], h0h[:, h1t, :],
                                start=(h1t == 0), stop=False,
                            )
                            nc.tensor.matmul(
                                ps[:], pw1h_s[:, h1t, hs], h0l[:, h1t, :],
                                start=False, stop=False,
                            )
                            nc.tensor.matmul(
                                ps[:], pw1l_s[:, h1t, hs], h0h[:, h1t, :],
                                start=False, stop=(h1t == HT - 1),
                            )
                        nc.scalar.activation(
                            h1[:, h2, :], ps[:], AF.Relu,
                            bias=pbb[:, 1, h2 : h2 + 1],
                        )
                    # l2: fp32 (exact h1 in SBUF). Compute logitsT [K, nb]
                    # with 512-moving matmuls (8 instead of 32 tiny ones),
                    # then tiny PE transposes back to row-major.
                    psT_full = ps_mm.tile([128, nb], F32, tag="mm")
                    psT = psT_full[0:K, :]
                    for ht in range(HT):
                        nc.tensor.matmul(
                            psT, pw2_s[:, ht, :], h1[:, ht, :],
                            start=(ht == 0), stop=(ht == HT - 1),
                        )
                    lT = ptmp.tile([K, nb], F32, tag="lT", bufs=2)
                    for t in range(ntl):
                        ts_ = slice(t * 128, (t + 1) * 128)
                        nc.vector.tensor_copy(lT[:, ts_], psT[:, ts_])
                    for t in range(ntl):
                        r = b * ntl + t
                        psl = ps_mm.tile([128, K], F32, tag="mm")
                        nc.tensor.transpose(
                            psl[:], lT[:, t * 128 : (t + 1) * 128],
                            ident[0:K, 0:K],
                        )
                        nc.vector.tensor_add(logits[:, r, :], psl[:], pb2_b)

            # ---- gumbel-max + softmax weights (off the PE critical path) ----
            nc.gpsimd.dma_start(
                out=gu, in_=gu_d.rearrange("(t p) k -> p t k", p=128)
            )
            # lg1 = log(u + EPS); then lg1 <- log(-lg1 + EPS) = -gumbel
            nc.scalar.activation(lg1, gu, AF.Ln, bias=eps_b)
            nc.scalar.activation(lg1, lg1, AF.Ln, bias=eps_b, scale=-1.0)
            # sc = logits + gumbel
            nc.vector.tensor_sub(sc, logits, lg1)
            with ExitStack() as ectx:
                ew = ectx.enter_context(tc.tile_pool(name="ew", bufs=1))
                eact = ectx.enter_context(tc.tile_pool(name="eact", bufs=1))
                tmp = ectx.enter_context(tc.tile_pool(name="etmp", bufs=1))
                nzp = ectx.enter_context(tc.tile_pool(name="nz", bufs=2))

                acc = eact.tile([128, rt, FDIM], F32, tag="acc")

                for r in range(rt):
                    # packed per-r temps: m1|mx|nmx|sm|rs | oh5 | ex | ps_t
                    tg = tmp.tile([128, 20], F32, tag="tg", bufs=2)
                    m1 = tg[:, 0:1]
                    mx = tg[:, 1:2]
                    nmx = tg[:, 2:3]
                    sm = tg[:, 3:4]
                    rs = tg[:, 4:5]
                    oh5 = tg[:, 5:10]
                    ex = tg[:, 10:15]
                    ps_t = tg[:, 15:20]
                    nc.vector.tensor_reduce(m1, sc[:, r, :], axis=AX.X, op=ALU.max)
                    nc.vector.tensor_scalar(
                        oh5, sc[:, r, :], m1, WEIGHT, ALU.is_ge, ALU.mult
                    )
                    nc.vector.tensor_reduce(
                        mx, logits[:, r, :], axis=AX.X, op=ALU.max
                    )
                    nc.vector.tensor_scalar_mul(nmx, mx, -1.0)
                    nc.scalar.activation(ex, logits[:, r, :], AF.Exp, bias=nmx)
                    nc.vector.tensor_reduce(sm, ex, axis=AX.X, op=ALU.add)
                    nc.vector.reciprocal(rs, sm)
                    nc.vector.tensor_scalar_mul(ps_t, ex, rs)
                    nc.vector.tensor_add(ps_t, ps_t, oh5)
                    nc.vector.tensor_scalar_mul(
                        wgt[:, r, :], ps_t, 1.0 / (1.0 + WEIGHT)
                    )

                # ---- experts ----
                for k in range(K):
                    if k == 0:
                        gw0_s, gw1_s, gw2_s, gbb, bb = gw0_f, gw1_f, gw2_f, gbb_f, bb_f
                    else:
                        gw0_s = ew.tile([128, CT, HDIM], BF16, tag="gw0", bufs=1)
                        nc.sync.dma_start(out=gw0_s, in_=gw0_d[k])
                        gw1_s = ew.tile([128, HT, HDIM], BF16, tag="gw1", bufs=2)
                        nc.sync.dma_start(out=gw1_s, in_=gw1_d[k])
                        gw2_s = ew.tile([128, HT, F2], BF16, tag="gw2", bufs=2)
                        nc.sync.dma_start(out=gw2_s, in_=gw2_d[k])
                        gbb = ew.tile([128, 2, HT], F32, tag="gbb", bufs=2)
                        nc.gpsimd.dma_start(out=gbb, in_=gbb_d[k])
                        bb = ew.tile([128, FDIM], F32, tag="bb", bufs=2)
                        nc.gpsimd.dma_start(
                            out=bb,
                            in_=gb2_d[k, 0:FDIM].partition_broadcast(128),
                        )

                    for b in range(nbc):
                        cs = slice(b * nb, (b + 1) * nb)
                        g0 = eact.tile([128, HT, nb], BF16, tag="a0")
                        for ht in range(HT):
                            ps = ps_mm.tile([128, nb], F32, tag="mm")
                            for ci in range(CT):
                                nc.tensor.matmul(
                                    ps[:],
                                    gw0_s[:, ci, ht * 128 : (ht + 1) * 128],
                                    chb[:, ci, cs],
                                    start=(ci == 0),
                                    stop=(ci == CT - 1),
                                )
                            nc.scalar.activation(
                                g0[:, ht, :], ps[:], AF.Relu,
                                bias=gbb[:, 0, ht : ht + 1],
                            )
                        g1 = eact.tile([128, HT, nb], BF16, tag="a1")
                        for h2 in range(HT):
                            ps = ps_mm.tile([128, nb], F32, tag="mm")
                            for h_1 in range(HT):
                                nc.tensor.matmul(
                                    ps[:],
                                    gw1_s[:, h_1, h2 * 128 : (h2 + 1) * 128],
                                    g0[:, h_1, :],
                                    start=(h_1 == 0),
                                    stop=(h_1 == HT - 1),
                                )
                            nc.scalar.activation(
                                g1[:, h2, :], ps[:], AF.Relu,
                                bias=gbb[:, 1, h2 : h2 + 1],
                            )
                        # layer 3: row-major output [n, 2F]. Row-tiles are
                        # processed in pairs with the matmul groups ordered
                        # m(t0), m(t1), lv(t0), lv(t1): the PE's moving
                        # stream then switches between the two gw2 halves
                        # half as often (each switch showed as a ~379 ns
                        # first matmul, once per row-tile-expert).
                        for tp in range(0, ntl, 2):
                            pair = (tp, tp + 1)
                            ps_ms = {}
                            ps_lvs = {}
                            for t in pair:
                                ts_ = slice(t * 128, (t + 1) * 128)
                                ps_m = ps_l3.tile(
                                    [128, FDIM], F32, tag="m", bufs=2
                                )
                                ps_ms[t] = ps_m
                                for ht in range(HT):
                                    nc.tensor.matmul(
                                        ps_m[:],
                                        g1[:, ht, ts_],
                                        gw2_s[:, ht, 0:FDIM],
                                        start=(ht == 0),
                                        stop=(ht == HT - 1),
                                    )
                            for t in pair:
                                ts_ = slice(t * 128, (t + 1) * 128)
                                ps_lv = ps_l3.tile(
                                    [128, FDIM], F32, tag="lv", bufs=2
                                )
                                ps_lvs[t] = ps_lv
                                for ht in range(HT):
                                    nc.tensor.matmul(
                                        ps_lv[:],
                                        g1[:, ht, ts_],
                                        gw2_s[:, ht, FDIM:F2],
                                        start=(ht == 0),
                                        stop=(ht == HT - 1),
                                    )
                            for t in pair:
                                r = b * ntl + t
                                # NB: gpsimd has no PSUM port — these must
                                # stay on vector/scalar. The logvar bias is
                                # host-folded into the noise
                                # (noise *= exp(0.5*gb2_lv)), so the scalar
                                # engine exps PSUM directly.
                                o_m = tmp.tile([128, FDIM], F32, tag="o_m", bufs=2)
                                nc.vector.tensor_add(o_m[:], ps_ms[t][:], bb)
                                std = tmp.tile([128, FDIM], F32, tag="std", bufs=2)
                                nc.scalar.activation(
                                    std[:], ps_lvs[t][:], AF.Exp, scale=0.5
                                )
                                nz_t = nzp.tile([128, FDIM], F32, tag="nz")
                                nc.sync.dma_start(
                                    out=nz_t,
                                    in_=noise_d[k, r * 128 : (r + 1) * 128, :],
                                )
                                # NB: keep the whole chain on DVE — splitting
                                # it across engines stalls the in-order DVE
                                # queue on cross-engine waits, delaying the
                                # PSUM drains.
                                nc.vector.tensor_mul(nz_t[:], nz_t[:], std[:])
                                nc.vector.tensor_add(nz_t[:], nz_t[:], o_m[:])
                                wv = wgt[:, r, k : k + 1]
                                if k == 0:
                                    nc.vector.tensor_scalar_mul(
                                        acc[:, r, :], nz_t[:], wv
                                    )
                                else:
                                    nc.vector.scalar_tensor_tensor(
                                        acc[:, r, :], nz_t[:], wv, acc[:, r, :],
                                        ALU.mult, ALU.add,
                                    )
                                if k == K - 1:
                                    # row r is final: ship it while later
                                    # rows are still computing
                                    nc.sync.dma_start(
                                        out=out_d[r * 128 : (r + 1) * 128, :],
                                        in_=acc[:, r, :],
                                    )
    nc.compile()
    return nc


_PROGRAM_CACHE = {}


def get_program(nl: int):
    if nl not in _PROGRAM_CACHE:
        _PROGRAM_CACHE[nl] = build_program(nl)
    return _PROGRAM_CACHE[nl]


def _split_bf16(x: np.ndarray):
    """Split fp32 array into bf16 hi (RTN) + bf16 residual lo."""
    import ml_dtypes

    x = np.ascontiguousarray(x, dtype=np.float32)
    hi = x.astype(ml_dtypes.bfloat16)
    lo = (x - hi.astype(np.float32)).astype(ml_dtypes.bfloat16)
    return np.ascontiguousarray(hi), np.ascontiguousarray(lo)


def _parr(w: np.ndarray, t: int):
    """[t*128, F] -> partition-contiguous [128, t, F]."""
    f = w.shape[-1]
    return np.ascontiguousarray(w.reshape(t, 128, f).transpose(1, 0, 2))


def make_in_maps(inputs: dict, n_cores: int = N_CORES):
    import ml_dtypes

    nl = inputs["c"].shape[0] // n_cores
    shared = {"pb2": np.ascontiguousarray(np.asarray(inputs["pb2"], np.float32))}
    shared["gb2"] = np.ascontiguousarray(np.asarray(inputs["gb2"], np.float32))
    pw0h, pw0l = _split_bf16(np.asarray(inputs["pw0"]))
    pw1h, pw1l = _split_bf16(np.asarray(inputs["pw1"]))
    shared["pw0h"] = _parr(pw0h, CT)
    shared["pw0l"] = _parr(pw0l, CT)
    shared["pw1h"] = _parr(pw1h, HT)
    shared["pw1l"] = _parr(pw1l, HT)
    shared["pw2"] = _parr(np.asarray(inputs["pw2"], np.float32), HT)
    # pbb: [128, 2, HT] packing of pb0|pb1 with (t p) -> p t layout
    pbb = np.stack(
        [
            np.asarray(inputs["pb0"], np.float32).reshape(HT, 128).T,
            np.asarray(inputs["pb1"], np.float32).reshape(HT, 128).T,
        ],
        axis=1,
    )
    shared["pbb"] = np.ascontiguousarray(pbb)
    gbb = np.stack(
        [
            np.asarray(inputs["gb0"], np.float32).reshape(K, HT, 128).transpose(0, 2, 1),
            np.asarray(inputs["gb1"], np.float32).reshape(K, HT, 128).transpose(0, 2, 1),
        ],
        axis=2,
    )
    shared["gbb"] = np.ascontiguousarray(gbb)  # [K, 128, 2, HT]
    for name, t in (("gw0", CT), ("gw1", HT), ("gw2", HT)):
        w = np.asarray(inputs[name], np.float32).astype(ml_dtypes.bfloat16)
        shared[name] = np.ascontiguousarray(
            w.reshape(K, t, 128, w.shape[-1]).transpose(0, 2, 1, 3)
        )
    c = np.asarray(inputs["c"], dtype=np.float32)
    # fold the logvar bias into the noise: noise_k *= exp(0.5 * gb2_lv_k)
    noise = np.asarray(inputs["noise"], dtype=np.float32) * np.exp(
        0.5 * np.asarray(inputs["gb2"], np.float32)[:, None, FDIM:]
    )
    noise = noise.astype(np.float32)
    gu = np.asarray(inputs["gumbel_u"], dtype=np.float32)
    in_maps = []
    for i in range(n_cores):
        rows = slice(i * nl, (i + 1) * nl)
        m = dict(shared)
        chb, crb = _split_bf16(c[rows].T)
        m["chb"] = _parr(chb, CT)
        m["crb"] = _parr(crb, CT)
        m["noise"] = np.ascontiguousarray(noise[:, rows, :])
        m["gumbel_u"] = np.ascontiguousarray(gu[rows])
        in_maps.append(m)
    return in_maps


def kernel(**inputs) -> np.ndarray:
    nc = get_program(N // N_CORES)
    in_maps = make_in_maps(inputs)
    res = run_bass_kernel_spmd(nc, in_maps, core_ids=list(range(N_CORES)))
    return np.concatenate(
        [res.results[i]["out"] for i in range(N_CORES)], axis=0
    )


# revision 53
# speedup vs baseline: 1.0040x; 1.0040x over previous
"""Trainium2 Bass kernel for nn_GaussianMixture (mixture-of-5-Gaussians sampler).

Strategy: data-parallel over the row dim N=16384 across 8 NeuronCores
(2048 rows/core), MLP weights replicated. Single pass over all 2048 rows
per core.

The probs MLP needs near-fp32 logits (the Gumbel argmax flips components
for logit errors ~1e-4, and each flip costs O(1) error on that row), so
its two hidden layers run as 3-matmul bf16 hi/lo splits (Whi*xhi +
Whi*xlo + Wlo*xhi, ~17 effective mantissa bits; measured logit absmax
err 3.7e-5 vs a 1.24e-4 min top-2 score margin on this dataset). c and
the probs weights arrive from the host pre-transposed and pre-split into
bf16 hi + bf16 residual parts; the hidden activation h0 is split
on-device with a bf16 round + subtract.

The expert MLPs run plain bf16 (measured ~216 ns per 512-row matmul vs
233 ns for f32r on this silicon; output error stays ~3e-3 << the 2e-2
budget, and expert precision cannot flip the component choice). The
experts reuse the bf16 hi part of c as their input.

All weights arrive host-pre-arranged into partition-contiguous
[128, ...] layout so every weight DMA is a trivial descriptor — complex
rearrange patterns cost the SP engine ~10us of serial descriptor
generation per expert, which showed up as PE stalls. Expert 0's full
weight set is prefetched during probs.

Activations stay feature-major ([h_features, n_rows]) through the hidden
layers so no transposes are needed; the final expert layer uses the
feature-major hidden state as lhsT to produce row-major output directly,
so noise / sampling / weighted-combine all run row-major with
per-partition scalar weights. Each row-tile's output is DMA'd out as
soon as the last expert's contribution lands.
"""
import sys

sys.path.insert(0, "/opt/trn_rl_repo")

from contextlib import ExitStack

import numpy as np

import concourse.bass as bass
import concourse.tile as tile
from concourse import bacc, mybir
from concourse.bass_utils import run_bass_kernel_spmd
from concourse.masks import make_identity

F32 = mybir.dt.float32
F32R = mybir.dt.float32r
BF16 = mybir.dt.bfloat16
AF = mybir.ActivationFunctionType
ALU = mybir.AluOpType
AX = mybir.AxisListType

N_CORES = 8
N, CDIM, FDIM, HDIM, K = 16384, 512, 512, 1024, 5
F2 = 2 * FDIM
WEIGHT = 5.0
EPS = 1e-20

CT = CDIM // 128  # 4 c-feature tiles
HT = HDIM // 128  # 8 h-feature tiles


def build_program(nl: int):
    """Build the per-core program for nl rows (nl=2048 for the real run)."""
    assert nl % 512 == 0
    nb = 512              # n-block (matmul moving size)
    nbc = nl // nb        # n-blocks
    ntl = nb // 128       # row-tiles per n-block
    rt = nl // 128        # row-tiles total

    nc = bacc.Bacc("TRN2", target_bir_lowering=False, debug=False)

    # All weight/bias tensors are host-pre-arranged to [128, ...]
    # partition-contiguous layout (see make_in_maps).
    chb_d = nc.dram_tensor("chb", [128, CT, nl], BF16, kind="ExternalInput").ap()
    crb_d = nc.dram_tensor("crb", [128, CT, nl], BF16, kind="ExternalInput").ap()
    noise_d = nc.dram_tensor("noise", [K, nl, FDIM], F32, kind="ExternalInput").ap()
    gu_d = nc.dram_tensor("gumbel_u", [nl, K], F32, kind="ExternalInput").ap()
    pw0h_d = nc.dram_tensor("pw0h", [128, CT, HDIM], BF16, kind="ExternalInput").ap()
    pw0l_d = nc.dram_tensor("pw0l", [128, CT, HDIM], BF16, kind="ExternalInput").ap()
    pw1h_d = nc.dram_tensor("pw1h", [128, HT, HDIM], BF16, kind="ExternalInput").ap()
    pw1l_d = nc.dram_tensor("pw1l", [128, HT, HDIM], BF16, kind="ExternalInput").ap()
    pw2_d = nc.dram_tensor("pw2", [128, HT, K], F32, kind="ExternalInput").ap()
    pbb_d = nc.dram_tensor("pbb", [128, 2, HT], F32, kind="ExternalInput").ap()
    pb2_d = nc.dram_tensor("pb2", [K], F32, kind="ExternalInput").ap()
    gw0_d = nc.dram_tensor("gw0", [K, 128, CT, HDIM], BF16, kind="ExternalInput").ap()
    gw1_d = nc.dram_tensor("gw1", [K, 128, HT, HDIM], BF16, kind="ExternalInput").ap()
    gw2_d = nc.dram_tensor("gw2", [K, 128, HT, F2], BF16, kind="ExternalInput").ap()
    gbb_d = nc.dram_tensor("gbb", [K, 128, 2, HT], F32, kind="ExternalInput").ap()
    gb2_d = nc.dram_tensor("gb2", [K, F2], F32, kind="ExternalInput").ap()
    out_d = nc.dram_tensor("out", [nl, FDIM], F32, kind="ExternalOutput").ap()

    with tile.TileContext(nc) as tc:
        with ExitStack() as gctx:
            const = gctx.enter_context(tc.tile_pool(name="const", bufs=1))
            ps_mm = gctx.enter_context(
                tc.tile_pool(name="ps_mm", bufs=4, space="PSUM")
            )
            ps_l3 = gctx.enter_context(
                tc.tile_pool(name="ps_l3", bufs=2, space="PSUM")
            )
            sb = gctx.enter_context(tc.tile_pool(name="sb", bufs=1))
            pre = gctx.enter_context(tc.tile_pool(name="pre", bufs=1))

            # packed const tile: identity | pb2 broadcast | eps
            constt = const.tile([128, 134], F32, tag="constt")
            ident = constt[:, 0:128]
            pb2_b = constt[:, 128:133]
            eps_b = constt[:, 133:134]
            make_identity(nc, ident)
            nc.gpsimd.dma_start(out=pb2_b, in_=pb2_d.partition_broadcast(128))
            nc.vector.memset(eps_b, EPS)

            # bf16 hi part of c.T: probs l0 input AND the experts' c input.
            # First block arrives as per-ci slices so the very first matmul
            # group can start as soon as ~0.4 MB has landed.
            chb = sb.tile([128, CT, nl], BF16, tag="chb")

            # smalls: logits | gu | lg1 | sc | wgt
            smalls = sb.tile([128, 5, rt, K], F32, tag="smalls")
            logits = smalls[:, 0]
            gu = smalls[:, 1]
            lg1 = smalls[:, 2]
            sc = smalls[:, 3]
            wgt = smalls[:, 4]

            # ---- probs MLP (bf16 hi/lo split, ~fp32-exact logits) ----
            with ExitStack() as pctx:
                pw = pctx.enter_context(tc.tile_pool(name="pw", bufs=1))
                act = pctx.enter_context(tc.tile_pool(name="pact", bufs=1))
                ptmp = pctx.enter_context(tc.tile_pool(name="ptmp", bufs=1))

                # DMA issue order tracks first use: block-0 inputs + l0
                # weights (per-ci so the first group streams), l1 weights,
                # the rest of c, then the expert-0 prefetch.
                crb = pw.tile([128, CT, nl], BF16, tag="crb")
                pw0h_s = pw.tile([128, CT, HDIM], BF16, tag="w0h")
                pw0l_s = pw.tile([128, CT, HDIM], BF16, tag="w0l")
                for ci in range(CT):
                    nc.sync.dma_start(
                        out=chb[:, ci, 0:nb], in_=chb_d[:, ci, 0:nb]
                    )
                    nc.sync.dma_start(
                        out=pw0h_s[:, ci, :], in_=pw0h_d[:, ci, :]
                    )
                    nc.sync.dma_start(
                        out=crb[:, ci, 0:nb], in_=crb_d[:, ci, 0:nb]
                    )
                    nc.sync.dma_start(
                        out=pw0l_s[:, ci, :], in_=pw0l_d[:, ci, :]
                    )
                pw1h_s = pw.tile([128, HT, HDIM], BF16, tag="w1h")
                nc.sync.dma_start(out=pw1h_s, in_=pw1h_d)
                pw1l_s = pw.tile([128, HT, HDIM], BF16, tag="w1l")
                nc.sync.dma_start(out=pw1l_s, in_=pw1l_d)
                for b in range(1, nbc):
                    cs = slice(b * nb, (b + 1) * nb)
                    nc.sync.dma_start(out=chb[:, :, cs], in_=chb_d[:, :, cs])
                    nc.sync.dma_start(out=crb[:, :, cs], in_=crb_d[:, :, cs])
                pw2_s = pw.tile([128, HT, K], F32, tag="w2")
                nc.gpsimd.dma_start(out=pw2_s, in_=pw2_d)
                pbb = pw.tile([128, 2, HT], F32, tag="pbb")
                nc.gpsimd.dma_start(out=pbb, in_=pbb_d)
                # expert 0's full weight set: descriptors + transfer hidden
                # under probs compute
                gw0_f = pre.tile([128, CT, HDIM], BF16, tag="gw0f")
                nc.sync.dma_start(out=gw0_f, in_=gw0_d[0])
                gw1_f = pre.tile([128, HT, HDIM], BF16, tag="gw1f")
                nc.sync.dma_start(out=gw1_f, in_=gw1_d[0])
                gw2_f = pre.tile([128, HT, F2], BF16, tag="gw2f")
                nc.sync.dma_start(out=gw2_f, in_=gw2_d[0])
                gbb_f = pre.tile([128, 2, HT], F32, tag="gbbf")
                nc.sync.dma_start(out=gbb_f, in_=gbb_d[0])
                bb_f = pre.tile([128, FDIM], F32, tag="bbf")
                nc.sync.dma_start(
                    out=bb_f, in_=gb2_d[0, 0:FDIM].partition_broadcast(128)
                )

                def emit_l2_tail(lT_prev, b_prev):
                    # transposes of the previous block's logitsT: deferred
                    # into the next block's l0 phase so the DVE lT copies
                    # have ~20us of cover instead of being waited on at
                    # the point of use
                    for t in range(ntl):
                        r = b_prev * ntl + t
                        psl = ps_mm.tile([128, K], F32, tag="mm")
                        nc.tensor.transpose(
                            psl[:], lT_prev[:, t * 128 : (t + 1) * 128],
                            ident[0:K, 0:K],
                        )
                        nc.vector.tensor_add(logits[:, r, :], psl[:], pb2_b)

                pending = None
                for b in range(nbc):
                    cs = slice(b * nb, (b + 1) * nb)
                    h0h = act.tile([128, HT, nb], BF16, tag="h0h")
                    h0l = act.tile([128, HT, nb], BF16, tag="h0l")
                    for ht in range(HT):
                        hs = slice(ht * 128, (ht + 1) * 128)
                        ps = ps_mm.tile([128, nb], F32, tag="mm")
                        for ci in range(CT):
                            nc.tensor.matmul(
                                ps[:], pw0h_s[:, ci, hs], chb[:, ci, cs],
                                start=(ci == 0), stop=False,
                            )
                            nc.tensor.matmul(
                                ps[:], pw0h_s[:, ci, hs], crb[:, ci, cs],
                                start=False, stop=False,
                            )
                            nc.tensor.matmul(
                                ps[:], pw0l_s[:, ci, hs], chb[:, ci, cs],
                                start=False, stop=(ci == CT - 1),
                            )
                        # relu in fp32, then split: hi = bf16(h0), lo = bf16(h0 - hi)
                        h0t = ptmp.tile([128, nb], F32, tag="h0t", bufs=2)
                        nc.scalar.activation(
                            h0t[:], ps[:], AF.Relu, bias=pbb[:, 0, ht : ht + 1]
                        )
                        nc.vector.tensor_copy(h0h[:, ht, :], h0t[:])
                        nc.gpsimd.tensor_sub(h0l[:, ht, :], h0t[:], h0h[:, ht, :])
                    if pending is not None:
                        emit_l2_tail(*pending)
                        pending = None
                    h1 = act.tile([128, HT, nb], F32, tag="h1")
                    for h2 in range(HT):
                        hs = slice(h2 * 128, (h2 + 1) * 128)
                        ps = ps_mm.tile([128, nb], F32, tag="mm")
                        for h1t in range(HT):
                            nc.tensor.matmul(
                                ps[:], pw1h_s[:, h1t, hs], h0h[:, h1t, :],
                                start=(h1t == 0), stop=False,
                            )
                            nc.tensor.matmul(
                                ps[:], pw1h_s[:, h1t, hs], h0l[:, h1t, :],
                                start=False, stop=False,
                            )
                            nc.tensor.matmul(
                                ps[:], pw1l_s[:, h1t, hs], h0h[:, h1t, :],
                                start=False, stop=(h1t == HT - 1),
                            )
                        nc.scalar.activation(
                            h1[:, h2, :], ps[:], AF.Relu,
                            bias=pbb[:, 1, h2 : h2 + 1],
                        )
                    # l2: fp32 (exact h1 in SBUF). Compute logitsT [K, nb]
                    # with 512-moving matmuls (8 instead of 32 tiny ones),
                    # then tiny PE transposes back to row-major.
                    psT_full = ps_mm.tile([128, nb], F32, tag="mm")
                    psT = psT_full[0:K, :]
                    for ht in range(HT):
                        nc.tensor.matmul(
                            psT, pw2_s[:, ht, :], h1[:, ht, :],
                            start=(ht == 0), stop=(ht == HT - 1),
                        )
                    lT = ptmp.tile([K, nb], F32, tag="lT", bufs=2)
                    for t in range(ntl):
                        ts_ = slice(t * 128, (t + 1) * 128)
                        nc.vector.tensor_copy(lT[:, ts_], psT[:, ts_])
                    pending = (lT, b)
                if pending is not None:
                    emit_l2_tail(*pending)

            # ---- gumbel-max + softmax weights (off the PE critical path) ----
            nc.gpsimd.dma_start(
                out=gu, in_=gu_d.rearrange("(t p) k -> p t k", p=128)
            )
            # lg1 = log(u + EPS); then lg1 <- log(-lg1 + EPS) = -gumbel
            nc.scalar.activation(lg1, gu, AF.Ln, bias=eps_b)
            nc.scalar.activation(lg1, lg1, AF.Ln, bias=eps_b, scale=-1.0)
            # sc = logits + gumbel
            nc.vector.tensor_sub(sc, logits, lg1)
            with ExitStack() as ectx:
                ew = ectx.enter_context(tc.tile_pool(name="ew", bufs=1))
                eact = ectx.enter_context(tc.tile_pool(name="eact", bufs=1))
                tmp = ectx.enter_context(tc.tile_pool(name="etmp", bufs=1))
                nzp = ectx.enter_context(tc.tile_pool(name="nz", bufs=2))

                acc = eact.tile([128, rt, FDIM], F32, tag="acc")

                for r in range(rt):
                    # packed per-r temps: m1|mx|nmx|sm|rs | oh5 | ex | ps_t
                    tg = tmp.tile([128, 20], F32, tag="tg", bufs=2)
                    m1 = tg[:, 0:1]
                    mx = tg[:, 1:2]
                    nmx = tg[:, 2:3]
                    sm = tg[:, 3:4]
                    rs = tg[:, 4:5]
                    oh5 = tg[:, 5:10]
                    ex = tg[:, 10:15]
                    ps_t = tg[:, 15:20]
                    nc.vector.tensor_reduce(m1, sc[:, r, :], axis=AX.X, op=ALU.max)
                    nc.vector.tensor_scalar(
                        oh5, sc[:, r, :], m1, WEIGHT, ALU.is_ge, ALU.mult
                    )
                    nc.vector.tensor_reduce(
                        mx, logits[:, r, :], axis=AX.X, op=ALU.max
                    )
                    nc.vector.tensor_scalar_mul(nmx, mx, -1.0)
                    nc.scalar.activation(ex, logits[:, r, :], AF.Exp, bias=nmx)
                    nc.vector.tensor_reduce(sm, ex, axis=AX.X, op=ALU.add)
                    nc.vector.reciprocal(rs, sm)
                    nc.vector.tensor_scalar_mul(ps_t, ex, rs)
                    nc.vector.tensor_add(ps_t, ps_t, oh5)
                    nc.vector.tensor_scalar_mul(
                        wgt[:, r, :], ps_t, 1.0 / (1.0 + WEIGHT)
                    )

                # ---- experts ----
                for k in range(K):
                    if k == 0:
                        gw0_s, gw1_s, gw2_s, gbb, bb = gw0_f, gw1_f, gw2_f, gbb_f, bb_f
                    else:
                        gw0_s = ew.tile([128, CT, HDIM], BF16, tag="gw0", bufs=1)
                        nc.sync.dma_start(out=gw0_s, in_=gw0_d[k])
                        gw1_s = ew.tile([128, HT, HDIM], BF16, tag="gw1", bufs=2)
                        nc.sync.dma_start(out=gw1_s, in_=gw1_d[k])
                        gw2_s = ew.tile([128, HT, F2], BF16, tag="gw2", bufs=2)
                        nc.sync.dma_start(out=gw2_s, in_=gw2_d[k])
                        gbb = ew.tile([128, 2, HT], F32, tag="gbb", bufs=2)
                        nc.gpsimd.dma_start(out=gbb, in_=gbb_d[k])
                        bb = ew.tile([128, FDIM], F32, tag="bb", bufs=2)
                        nc.gpsimd.dma_start(
                            out=bb,
                            in_=gb2_d[k, 0:FDIM].partition_broadcast(128),
                        )

                    for b in range(nbc):
                        cs = slice(b * nb, (b + 1) * nb)
                        g0 = eact.tile([128, HT, nb], BF16, tag="a0")
                        for ht in range(HT):
                            ps = ps_mm.tile([128, nb], F32, tag="mm")
                            for ci in range(CT):
                                nc.tensor.matmul(
                                    ps[:],
                                    gw0_s[:, ci, ht * 128 : (ht + 1) * 128],
                                    chb[:, ci, cs],
                                    start=(ci == 0),
                                    stop=(ci == CT - 1),
                                )
                            nc.scalar.activation(
                                g0[:, ht, :], ps[:], AF.Relu,
                                bias=gbb[:, 0, ht : ht + 1],
                            )
                        g1 = eact.tile([128, HT, nb], BF16, tag="a1")
                        for h2 in range(HT):
                            ps = ps_mm.tile([128, nb], F32, tag="mm")
                            for h_1 in range(HT):
                                nc.tensor.matmul(
                                    ps[:],
                                    gw1_s[:, h_1, h2 * 128 : (h2 + 1) * 128],
                                    g0[:, h_1, :],
                                    start=(h_1 == 0),
                                    stop=(h_1 == HT - 1),
                                )
                            nc.scalar.activation(
                                g1[:, h2, :], ps[:], AF.Relu,
                                bias=gbb[:, 1, h2 : h2 + 1],
                            )
                        # layer 3: row-major output [n, 2F]
                        for t in range(ntl):
                            r = b * ntl + t
                            ts_ = slice(t * 128, (t + 1) * 128)
                            # tag m is 3-deep: the DVE o_m add runs ~1
                            # row-tile behind and collides with the t+2
                            # matmul writes at 2-deep (PSUM bank conflict,
                            # measured as 379 ns matmuls)
                            ps_m = ps_l3.tile([128, FDIM], F32, tag="m", bufs=2)
                            ps_lv = ps_l3.tile([128, FDIM], F32, tag="lv", bufs=2)
                            for ht in range(HT):
                                nc.tensor.matmul(
                                    ps_m[:],
                                    g1[:, ht, ts_],
                                    gw2_s[:, ht, 0:FDIM],
                                    start=(ht == 0),
                                    stop=(ht == HT - 1),
                                )
                            for ht in range(HT):
                                nc.tensor.matmul(
                                    ps_lv[:],
                                    g1[:, ht, ts_],
                                    gw2_s[:, ht, FDIM:F2],
                                    start=(ht == 0),
                                    stop=(ht == HT - 1),
                                )
                            # NB: gpsimd has no PSUM port — these must stay
                            # on vector/scalar. The logvar bias is host-folded
                            # into the noise (noise *= exp(0.5*gb2_lv)), so
                            # the scalar engine exps PSUM directly and the
                            # DVE chain has slack vs the PE in this phase.
                            o_m = tmp.tile([128, FDIM], F32, tag="o_m", bufs=2)
                            nc.vector.tensor_add(o_m[:], ps_m[:], bb)
                            std = tmp.tile([128, FDIM], F32, tag="std", bufs=2)
                            nc.scalar.activation(std[:], ps_lv[:], AF.Exp, scale=0.5)
                            nz_t = nzp.tile([128, FDIM], F32, tag="nz")
                            nc.sync.dma_start(
                                out=nz_t,
                                in_=noise_d[k, r * 128 : (r + 1) * 128, :],
                            )
                            # NB: keep the whole chain on DVE — splitting it
                            # across engines stalls the in-order DVE queue on
                            # cross-engine waits, delaying the PSUM drains.
                            nc.vector.tensor_mul(nz_t[:], nz_t[:], std[:])
                            nc.vector.tensor_add(nz_t[:], nz_t[:], o_m[:])
                            wv = wgt[:, r, k : k + 1]
                            if k == 0:
                                nc.vector.tensor_scalar_mul(acc[:, r, :], nz_t[:], wv)
                            else:
                                nc.vector.scalar_tensor_tensor(
                                    acc[:, r, :], nz_t[:], wv, acc[:, r, :],
                                    ALU.mult, ALU.add,
                                )
                            if k == K - 1:
                                # row r is final: ship it while later rows
                                # are still computing
                                nc.sync.dma_start(
                                    out=out_d[r * 128 : (r + 1) * 128, :],
                                    in_=acc[:, r, :],
                                )
    nc.compile()
    return nc


_PROGRAM_CACHE = {}


def get_program(nl: int):
    if nl not in _PROGRAM_CACHE:
        _PROGRAM_CACHE[nl] = build_program(nl)
    return _PROGRAM_CACHE[nl]


def _split_bf16(x: np.ndarray):
    """Split fp32 array into bf16 hi (RTN) + bf16 residual lo."""
    import ml_dtypes

    x = np.ascontiguousarray(x, dtype=np.float32)
    hi = x.astype(ml_dtypes.bfloat16)
    lo = (x - hi.astype(np.float32)).astype(ml_dtypes.bfloat16)
    return np.ascontiguousarray(hi), np.ascontiguousarray(lo)


def _parr(w: np.ndarray, t: int):
    """[t*128, F] -> partition-contiguous [128, t, F]."""
    f = w.shape[-1]
    return np.ascontiguousarray(w.reshape(t, 128, f).transpose(1, 0, 2))


def make_in_maps(inputs: dict, n_cores: int = N_CORES):
    import ml_dtypes

    nl = inputs["c"].shape[0] // n_cores
    shared = {"pb2": np.ascontiguousarray(np.asarray(inputs["pb2"], np.float32))}
    shared["gb2"] = np.ascontiguousarray(np.asarray(inputs["gb2"], np.float32))
    pw0h, pw0l = _split_bf16(np.asarray(inputs["pw0"]))
    pw1h, pw1l = _split_bf16(np.asarray(inputs["pw1"]))
    shared["pw0h"] = _parr(pw0h, CT)
    shared["pw0l"] = _parr(pw0l, CT)
    shared["pw1h"] = _parr(pw1h, HT)
    shared["pw1l"] = _parr(pw1l, HT)
    shared["pw2"] = _parr(np.asarray(inputs["pw2"], np.float32), HT)
    # pbb: [128, 2, HT] packing of pb0|pb1 with (t p) -> p t layout
    pbb = np.stack(
        [
            np.asarray(inputs["pb0"], np.float32).reshape(HT, 128).T,
            np.asarray(inputs["pb1"], np.float32).reshape(HT, 128).T,
        ],
        axis=1,
    )
    shared["pbb"] = np.ascontiguousarray(pbb)
    gbb = np.stack(
        [
            np.asarray(inputs["gb0"], np.float32).reshape(K, HT, 128).transpose(0, 2, 1),
            np.asarray(inputs["gb1"], np.float32).reshape(K, HT, 128).transpose(0, 2, 1),
        ],
        axis=2,
    )
    shared["gbb"] = np.ascontiguousarray(gbb)  # [K, 128, 2, HT]
    for name, t in (("gw0", CT), ("gw1", HT), ("gw2", HT)):
        w = np.asarray(inputs[name], np.float32).astype(ml_dtypes.bfloat16)
        shared[name] = np.ascontiguousarray(
            w.reshape(K, t, 128, w.shape[-1]).transpose(0, 2, 1, 3)
        )
    c = np.asarray(inputs["c"], dtype=np.float32)
    # fold the logvar bias into the noise: noise_k *= exp(0.5 * gb2_lv_k)
    noise = np.asarray(inputs["noise"], dtype=np.float32) * np.exp(
        0.5 * np.asarray(inputs["gb2"], np.float32)[:, None, FDIM:]
    )
    noise = noise.astype(np.float32)
    gu = np.asarray(inputs["gumbel_u"], dtype=np.float32)
    in_maps = []
    for i in range(n_cores):
        rows = slice(i * nl, (i + 1) * nl)
        m = dict(shared)
        chb, crb = _split_bf16(c[rows].T)
        m["chb"] = _parr(chb, CT)
        m["crb"] = _parr(crb, CT)
        m["noise"] = np.ascontiguousarray(noise[:, rows, :])
        m["gumbel_u"] = np.ascontiguousarray(gu[rows])
        in_maps.append(m)
    return in_maps


def kernel(**inputs) -> np.ndarray:
    nc = get_program(N // N_CORES)
    in_maps = make_in_maps(inputs)
    res = run_bass_kernel_spmd(nc, in_maps, core_ids=list(range(N_CORES)))
    return np.concatenate(
        [res.results[i]["out"] for i in range(N_CORES)], axis=0
    )
